# revision 2
# baseline (speedup 1.0000x reference)
import sys
import numpy as np

sys.path.insert(0, "/opt/trn_rl_repo")

D = 128
B = 64
LMAX = 256
AMAX = 1024
NCORES = 8
SLOTS = B // NCORES  # 8 segments per core

_CACHE = {}


def _chunks(cols):
    """Split cols (multiple of 128) into pieces <=512, each a multiple of 128,
    preferring pieces >=256 (fp32r full-rate needs moving dim >=256)."""
    out = []
    k = (cols + 511) // 512
    base = cols // k
    base -= base % 128
    rem = cols - base * k
    off = 0
    for i in range(k):
        w = base + (128 if i < rem // 128 else 0)
        out.append((off, w))
        off += w
    assert off == cols, (cols, out)
    return out


def _build(ta_slots):
    import concourse.bass as bass
    from concourse import bacc
    import concourse.mybir as mybir
    import concourse.tile as tile
    from concourse.masks import make_identity

    dt = mybir.dt
    AF = mybir.ActivationFunctionType
    OP = mybir.AluOpType

    nc = bacc.Bacc(None)

    aa_in = nc.dram_tensor("aa_in", [SLOTS, AMAX, D], dt.float32, kind="ExternalInput")
    lig_in = nc.dram_tensor("lig_in", [SLOTS, LMAX, D], dt.float32, kind="ExternalInput")
    wpack = nc.dram_tensor("wpack", [D, 7 * D], dt.float32, kind="ExternalInput")
    bcol = nc.dram_tensor("bcol", [D, 8], dt.float32, kind="ExternalInput")
    scal = nc.dram_tensor("scal", [D, 4 * SLOTS], dt.float32, kind="ExternalInput")
    amask = nc.dram_tensor("amask", [SLOTS * AMAX], dt.float32, kind="ExternalInput")
    lmask = nc.dram_tensor("lmask", [SLOTS * LMAX], dt.float32, kind="ExternalInput")
    aa_out = nc.dram_tensor("aa_out", [SLOTS, AMAX, D], dt.float32, kind="ExternalOutput")
    lig_out = nc.dram_tensor("lig_out", [SLOTS, LMAX, D], dt.float32, kind="ExternalOutput")

    f32 = dt.float32
    f32r = dt.float32r

    with tile.TileContext(nc) as tc:
        with (
            tc.tile_pool(name="const", bufs=1) as cpool,
            tc.tile_pool(name="io", bufs=2) as iop,
            tc.tile_pool(name="act", bufs=2) as ap,
            tc.tile_pool(name="ps", bufs=6, space="PSUM") as ps,
            tc.tile_pool(name="pt", bufs=2, space="PSUM") as pst,
        ):
            # ---- constants ----
            ident = cpool.tile([128, 128], f32)
            make_identity(nc, ident[:])
            wt = cpool.tile([D, 7 * D], f32r)
            nc.sync.dma_start(wt[:], wpack[:].bitcast(f32r))
            WQ = wt[:, 0 * D:1 * D]
            WK = wt[:, 1 * D:2 * D]
            WV = wt[:, 2 * D:3 * D]
            WR1 = wt[:, 3 * D:4 * D]
            WR2 = wt[:, 4 * D:5 * D]
            WL1 = wt[:, 5 * D:6 * D]
            WL2 = wt[:, 6 * D:7 * D]
            bt = cpool.tile([D, 8], f32)
            nc.sync.dma_start(bt[:], bcol[:])
            BQ = bt[:, 0:1]
            BK = bt[:, 1:2]
            BV = bt[:, 2:3]
            BR1 = bt[:, 3:4]
            BL1 = bt[:, 4:5]
            st = cpool.tile([D, 4 * SLOTS], f32)
            nc.sync.dma_start(st[:], scal[:])
            # col layout: [br2/n_l | bl2/n_a | 1/n_l | 1/n_a] per slot
            amt = cpool.tile([128, SLOTS * (AMAX // 128)], f32)
            nc.sync.dma_start(amt[:], amask.rearrange("(n p) -> p n", p=128))
            lmt = cpool.tile([128, SLOTS * (LMAX // 128)], f32)
            nc.sync.dma_start(lmt[:], lmask.rearrange("(n p) -> p n", p=128))

            for s in range(SLOTS):
                ta = ta_slots[s]
                acols = ta * 128
                ach = _chunks(acols)
                BR2s = st[:, 0 * SLOTS + s:0 * SLOTS + s + 1]
                BL2s = st[:, 1 * SLOTS + s:1 * SLOTS + s + 1]
                INVLs = st[:, 2 * SLOTS + s:2 * SLOTS + s + 1]
                INVAs = st[:, 3 * SLOTS + s:3 * SLOTS + s + 1]

                # ---- loads (token-major) ----
                AAtok = ap.tile([128, ta * D], f32, tag="aatok")
                nc.sync.dma_start(
                    AAtok[:].rearrange("p (c d) -> p c d", d=D),
                    aa_in[s, 0:acols, :].rearrange("(c p) d -> p c d", p=128),
                )
                LIGtok = ap.tile([128, 2 * D], f32, tag="ligtok")
                nc.sync.dma_start(
                    LIGtok[:].rearrange("p (c d) -> p c d", d=D),
                    lig_in[s].rearrange("(c p) d -> p c d", p=128),
                )

                # ---- transposes to feature-major ----
                aaT = ap.tile([128, acols], f32r, tag="aaT")
                for c in range(ta):
                    p = pst.tile([128, 128], f32, tag="tp")
                    nc.tensor.transpose(p[:], AAtok[:, c * D:(c + 1) * D], ident[:])
                    nc.vector.tensor_copy(aaT[:, c * 128:(c + 1) * 128], p[:])
                ligT = ap.tile([128, 2 * 128], f32r, tag="ligT")
                for c in range(2):
                    p = pst.tile([128, 128], f32, tag="tp")
                    nc.tensor.transpose(p[:], LIGtok[:, c * D:(c + 1) * D], ident[:])
                    nc.vector.tensor_copy(ligT[:, c * 128:(c + 1) * 128], p[:])

                # ---- attention 1 projections ----
                qT = ap.tile([128, acols], f32r, tag="qT")
                for off, w in ach:
                    p = ps.tile([128, w], f32, tag="mm")
                    nc.tensor.matmul(p[:], WQ, aaT[:, off:off + w])
                    nc.scalar.activation(qT[:, off:off + w], p[:], AF.Identity, bias=BQ)
                kT = ap.tile([128, 256], f32r, tag="kT")
                p = ps.tile([128, 256], f32, tag="mm")
                nc.tensor.matmul(p[:], WK, ligT[:])
                nc.scalar.activation(kT[:], p[:], AF.Identity, bias=BK)
                vT = ap.tile([128, 256], f32, tag="vT")
                p = ps.tile([128, 256], f32, tag="mm")
                nc.tensor.matmul(p[:], WV, ligT[:])
                nc.scalar.activation(vT[:], p[:], AF.Identity, bias=BV)
                Vtok = ap.tile([128, 2 * D], f32r, tag="Vtok")
                for c in range(2):
                    p = pst.tile([128, 128], f32, tag="tp")
                    nc.tensor.transpose(p[:], vT[:, c * D:(c + 1) * D], ident[:])
                    nc.vector.tensor_scalar(
                        Vtok[:, c * D:(c + 1) * D], p[:],
                        lmt[:, 2 * s + c:2 * s + c + 1], None, OP.mult,
                    )

                # ---- attention 1 scores + message ----
                ST = ap.tile([128, 2 * acols], f32r, tag="ST")
                for lc in range(2):
                    for off, w in ach:
                        p = ps.tile([128, w], f32, tag="mm")
                        nc.tensor.matmul(
                            p[:], kT[:, lc * 128:(lc + 1) * 128], qT[:, off:off + w]
                        )
                        nc.scalar.activation(
                            ST[:, lc * acols + off:lc * acols + off + w], p[:], AF.Sigmoid
                        )
                msgT = ap.tile([128, acols], f32r, tag="msgT")
                for off, w in ach:
                    p = ps.tile([128, w], f32, tag="mm")
                    for lc in range(2):
                        nc.tensor.matmul(
                            p[:], Vtok[:, lc * D:(lc + 1) * D],
                            ST[:, lc * acols + off:lc * acols + off + w],
                            start=(lc == 0), stop=(lc == 1),
                        )
                    nc.vector.tensor_copy(msgT[:, off:off + w], p[:])

                # ---- aa MLP ----
                hT = ap.tile([128, acols], f32r, tag="hT")
                for off, w in ach:
                    p = ps.tile([128, w], f32, tag="mm")
                    nc.tensor.matmul(p[:], WR1, msgT[:, off:off + w])
                    nc.scalar.activation(
                        hT[:, off:off + w], p[:], AF.Lrelu, bias=BR1, alpha=0.1
                    )
                mT = ap.tile([128, acols], f32, tag="mT")
                for off, w in ach:
                    p = ps.tile([128, w], f32, tag="mm")
                    nc.tensor.matmul(p[:], WR2, hT[:, off:off + w])
                    # Lrelu(x/n + b/n) = Lrelu(x + b)/n  (positive homogeneity)
                    nc.scalar.activation(
                        mT[:, off:off + w], p[:], AF.Lrelu,
                        bias=BR2s, scale=INVLs, alpha=0.1,
                    )
                aa_newT = ap.tile([128, acols], f32r, tag="aanewT")
                nc.vector.tensor_tensor(
                    aa_newT[:], mT[:], aaT[:].bitcast(f32), OP.add
                )

                # ---- aa output (token-major, residual from pristine load) ----
                AAout = ap.tile([128, ta * D], f32, tag="aaout")
                for c in range(ta):
                    p = pst.tile([128, 128], f32, tag="tp")
                    nc.tensor.transpose(p[:], mT[:, c * 128:(c + 1) * 128], ident[:])
                    nc.vector.tensor_tensor(
                        AAout[:, c * D:(c + 1) * D], p[:],
                        AAtok[:, c * D:(c + 1) * D], OP.add,
                    )
                    nc.vector.tensor_scalar(
                        AAout[:, c * D:(c + 1) * D], AAout[:, c * D:(c + 1) * D],
                        amt[:, 8 * s + c:8 * s + c + 1], None, OP.mult,
                    )
                nc.sync.dma_start(
                    aa_out[s, 0:acols, :].rearrange("(c p) d -> p c d", p=128),
                    AAout[:].rearrange("p (c d) -> p c d", d=D),
                )

                # ---- attention 2 projections ----
                q2T = ap.tile([128, 256], f32r, tag="q2T")
                p = ps.tile([128, 256], f32, tag="mm")
                nc.tensor.matmul(p[:], WQ, ligT[:])
                nc.scalar.activation(q2T[:], p[:], AF.Identity, bias=BQ)
                k2T = ap.tile([128, acols], f32r, tag="k2T")
                for off, w in ach:
                    p = ps.tile([128, w], f32, tag="mm")
                    nc.tensor.matmul(p[:], WK, aa_newT[:, off:off + w])
                    nc.scalar.activation(k2T[:, off:off + w], p[:], AF.Identity, bias=BK)
                v2T = ap.tile([128, acols], f32, tag="v2T")
                for off, w in ach:
                    p = ps.tile([128, w], f32, tag="mm")
                    nc.tensor.matmul(p[:], WV, aa_newT[:, off:off + w])
                    nc.scalar.activation(v2T[:, off:off + w], p[:], AF.Identity, bias=BV)
                V2tok = ap.tile([128, ta * D], f32r, tag="V2tok")
                for c in range(ta):
                    p = pst.tile([128, 128], f32, tag="tp")
                    nc.tensor.transpose(p[:], v2T[:, c * 128:(c + 1) * 128], ident[:])
                    nc.vector.tensor_scalar(
                        V2tok[:, c * D:(c + 1) * D], p[:],
                        amt[:, 8 * s + c:8 * s + c + 1], None, OP.mult,
                    )

                # ---- attention 2 scores + message ----
                S2T = ap.tile([128, ta * 256], f32r, tag="S2T")
                for ac in range(ta):
                    p = ps.tile([128, 256], f32, tag="mm")
                    nc.tensor.matmul(p[:], k2T[:, ac * 128:(ac + 1) * 128], q2T[:])
                    nc.scalar.activation(
                        S2T[:, ac * 256:(ac + 1) * 256], p[:], AF.Sigmoid
                    )
                msg2T = ap.tile([128, 256], f32r, tag="msg2T")
                p = ps.tile([128, 256], f32, tag="mm")
                for ac in range(ta):
                    nc.tensor.matmul(
                        p[:], V2tok[:, ac * D:(ac + 1) * D],
                        S2T[:, ac * 256:(ac + 1) * 256],
                        start=(ac == 0), stop=(ac == ta - 1),
                    )
                nc.vector.tensor_copy(msg2T[:], p[:])

                # ---- lig MLP ----
                h2T = ap.tile([128, 256], f32r, tag="h2T")
                p = ps.tile([128, 256], f32, tag="mm")
                nc.tensor.matmul(p[:], WL1, msg2T[:])
                nc.scalar.activation(h2T[:], p[:], AF.Lrelu, bias=BL1, alpha=0.1)
                m2T = ap.tile([128, 256], f32, tag="m2T")
                p = ps.tile([128, 256], f32, tag="mm")
                nc.tensor.matmul(p[:], WL2, h2T[:])
                nc.scalar.activation(
                    m2T[:], p[:], AF.Lrelu, bias=BL2s, scale=INVAs, alpha=0.1
                )

                # ---- lig output ----
                LOUT = ap.tile([128, 2 * D], f32, tag="lout")
                for c in range(2):
                    p = pst.tile([128, 128], f32, tag="tp")
                    nc.tensor.transpose(p[:], m2T[:, c * 128:(c + 1) * 128], ident[:])
                    nc.vector.tensor_tensor(
                        LOUT[:, c * D:(c + 1) * D], p[:],
                        LIGtok[:, c * D:(c + 1) * D], OP.add,
                    )
                    nc.vector.tensor_scalar(
                        LOUT[:, c * D:(c + 1) * D], LOUT[:, c * D:(c + 1) * D],
                        lmt[:, 2 * s + c:2 * s + c + 1], None, OP.mult,
                    )
                nc.sync.dma_start(
                    lig_out[s].rearrange("(c p) d -> p c d", p=128),
                    LOUT[:].rearrange("p (c d) -> p c d", d=D),
                )

    nc.compile()
    return nc


def _get_program(ta_slots):
    key = tuple(ta_slots)
    if key not in _CACHE:
        _CACHE[key] = _build(key)
    return _CACHE[key]


def kernel(ligand_features, aa_features, lig_len, aa_len,
           Wq, bq, Wk, bk, Wv, bv, Wr1, br1, Wr2, br2, Wl1, bl1, Wl2, bl2):
    from concourse.bass_utils import run_bass_kernel_spmd

    ligand_features = np.ascontiguousarray(np.asarray(ligand_features, dtype=np.float32))
    aa_features = np.ascontiguousarray(np.asarray(aa_features, dtype=np.float32))
    lig_len_np = np.asarray(lig_len).astype(np.int64)
    aa_len_np = np.asarray(aa_len).astype(np.int64)

    # segment -> (core, slot) assignment: sort by aa_len so each slot has
    # near-equal lengths across cores (slot shape = max over its 8 cores)
    order = np.argsort(aa_len_np, kind="stable")
    seg_of = order.reshape(SLOTS, NCORES)  # seg_of[s, c]
    ta_slots = [
        int(-(-int(aa_len_np[seg_of[s]].max()) // 128)) for s in range(SLOTS)
    ]

    nc = _get_program(ta_slots)

    wpack = np.concatenate(
        [np.asarray(w, np.float32) for w in (Wq, Wk, Wv, Wr1, Wr2, Wl1, Wl2)], axis=1
    )
    bcol = np.zeros((D, 8), np.float32)
    for i, b in enumerate((bq, bk, bv, br1, bl1)):
        bcol[:, i] = np.asarray(b, np.float32)
    br2 = np.asarray(br2, np.float32)
    bl2 = np.asarray(bl2, np.float32)

    ar = np.arange(AMAX)
    lr = np.arange(LMAX)

    in_maps = []
    for c in range(NCORES):
        segs = seg_of[:, c]
        nl = lig_len_np[segs].astype(np.float32)
        na = aa_len_np[segs].astype(np.float32)
        scal = np.zeros((D, 4 * SLOTS), np.float32)
        for s in range(SLOTS):
            scal[:, 0 * SLOTS + s] = br2 / nl[s]
            scal[:, 1 * SLOTS + s] = bl2 / na[s]
            scal[:, 2 * SLOTS + s] = 1.0 / nl[s]
            scal[:, 3 * SLOTS + s] = 1.0 / na[s]
        amask = (ar[None, :] < aa_len_np[segs][:, None]).astype(np.float32).ravel()
        lmask = (lr[None, :] < lig_len_np[segs][:, None]).astype(np.float32).ravel()
        in_maps.append({
            "aa_in": np.ascontiguousarray(aa_features[segs]),
            "lig_in": np.ascontiguousarray(ligand_features[segs]),
            "wpack": np.ascontiguousarray(wpack),
            "bcol": bcol,
            "scal": scal,
            "amask": amask,
            "lmask": lmask,
        })

    res = run_bass_kernel_spmd(nc, in_maps, core_ids=list(range(NCORES)))

    lig_full = np.zeros((B, LMAX, D), np.float32)
    aa_full = np.zeros((B, AMAX, D), np.float32)
    for c in range(NCORES):
        r = res.results[c]
        for s in range(SLOTS):
            seg = seg_of[s, c]
            lig_full[seg] = r["lig_out"][s]
            aa_full[seg] = r["aa_out"][s]
    return lig_full, aa_full


if __name__ == "__main__":
    rng = np.random.default_rng(0)
    inputs = dict(
        ligand_features=rng.standard_normal((B, LMAX, D), dtype=np.float32),
        aa_features=rng.standard_normal((B, AMAX, D), dtype=np.float32),
        lig_len=rng.integers(1, LMAX + 1, B).astype(np.int32),
        aa_len=rng.integers(1, AMAX + 1, B).astype(np.int32),
    )
    s = 1.0 / np.sqrt(D)
    for nm in ("Wq", "Wk", "Wv", "Wr1", "Wr2", "Wl1", "Wl2"):
        inputs[nm] = rng.uniform(-s, s, (D, D)).astype(np.float32)
    for nm in ("bq", "bk", "bv", "br1", "br2", "bl1", "bl2"):
        inputs[nm] = rng.uniform(-s, s, D).astype(np.float32)
    lig_o, aa_o = kernel(**inputs)
    print("shapes", lig_o.shape, aa_o.shape)


# revision 6
# speedup vs baseline: 1.2838x; 1.2838x over previous
import sys
import numpy as np

sys.path.insert(0, "/opt/trn_rl_repo")

D = 128
B = 64
LMAX = 256
AMAX = 1024
NCORES = 8
SLOTS = B // NCORES  # 8 segments per core

_CACHE = {}


def _chunks(cols):
    """Split cols (multiple of 128) into pieces <=512, each a multiple of 128,
    preferring pieces >=256 (fp32r full-rate needs moving dim >=256)."""
    out = []
    k = (cols + 511) // 512
    base = cols // k
    base -= base % 128
    rem = cols - base * k
    off = 0
    for i in range(k):
        w = base + (128 if i < rem // 128 else 0)
        out.append((off, w))
        off += w
    assert off == cols, (cols, out)
    return out


def _build(ta_slots):
    import concourse.bass as bass
    from concourse import bacc
    import concourse.mybir as mybir
    import concourse.tile as tile
    from concourse.masks import make_identity

    dt = mybir.dt
    AF = mybir.ActivationFunctionType
    OP = mybir.AluOpType

    nc = bacc.Bacc(None)

    aa_in = nc.dram_tensor("aa_in", [SLOTS, AMAX, D], dt.float32, kind="ExternalInput")
    lig_in = nc.dram_tensor("lig_in", [SLOTS, LMAX, D], dt.float32, kind="ExternalInput")
    wpack = nc.dram_tensor("wpack", [D, 7 * D], dt.float32, kind="ExternalInput")
    bcol = nc.dram_tensor("bcol", [D, 8], dt.float32, kind="ExternalInput")
    scal = nc.dram_tensor("scal", [D, 4 * SLOTS], dt.float32, kind="ExternalInput")
    amask = nc.dram_tensor("amask", [SLOTS * AMAX], dt.float32, kind="ExternalInput")
    lmask = nc.dram_tensor("lmask", [SLOTS * LMAX], dt.float32, kind="ExternalInput")
    aa_out = nc.dram_tensor("aa_out", [SLOTS, AMAX, D], dt.float32, kind="ExternalOutput")
    lig_out = nc.dram_tensor("lig_out", [SLOTS, LMAX, D], dt.float32, kind="ExternalOutput")

    f32 = dt.float32
    f32r = dt.float32r

    with tile.TileContext(nc) as tc:
        with (
            tc.tile_pool(name="const", bufs=1) as cpool,
            tc.tile_pool(name="io", bufs=2) as iop,
            tc.tile_pool(name="act", bufs=2) as ap,
            tc.tile_pool(name="ps", bufs=6, space="PSUM") as ps,
            tc.tile_pool(name="pt", bufs=2, space="PSUM") as pst,
        ):
            # ---- constants ----
            ident = cpool.tile([128, 128], f32)
            make_identity(nc, ident[:])
            wt = cpool.tile([D, 7 * D], f32r)
            nc.sync.dma_start(wt[:], wpack[:].bitcast(f32r))
            WQ = wt[:, 0 * D:1 * D]
            WK = wt[:, 1 * D:2 * D]
            WV = wt[:, 2 * D:3 * D]
            WR1 = wt[:, 3 * D:4 * D]
            WR2 = wt[:, 4 * D:5 * D]
            WL1 = wt[:, 5 * D:6 * D]
            WL2 = wt[:, 6 * D:7 * D]
            bt = cpool.tile([D, 8], f32)
            nc.sync.dma_start(bt[:], bcol[:])
            BQ = bt[:, 0:1]
            BK = bt[:, 1:2]
            BV = bt[:, 2:3]
            BR1 = bt[:, 3:4]
            BL1 = bt[:, 4:5]
            st = cpool.tile([D, 4 * SLOTS], f32)
            nc.sync.dma_start(st[:], scal[:])
            # col layout: [br2/n_l | bl2/n_a | 1/n_l | 1/n_a] per slot
            amt = cpool.tile([128, SLOTS * (AMAX // 128)], f32)
            nc.sync.dma_start(amt[:], amask.rearrange("(n p) -> p n", p=128))
            lmt = cpool.tile([128, SLOTS * (LMAX // 128)], f32)
            nc.sync.dma_start(lmt[:], lmask.rearrange("(n p) -> p n", p=128))

            for s in range(SLOTS):
                ta = ta_slots[s]
                acols = ta * 128
                ach = _chunks(acols)
                BR2s = st[:, 0 * SLOTS + s:0 * SLOTS + s + 1]
                BL2s = st[:, 1 * SLOTS + s:1 * SLOTS + s + 1]
                INVLs = st[:, 2 * SLOTS + s:2 * SLOTS + s + 1]
                INVAs = st[:, 3 * SLOTS + s:3 * SLOTS + s + 1]

                # ---- loads (token-major) ----
                AAtok = ap.tile([128, ta * D], f32, tag="aatok")
                nc.sync.dma_start(
                    AAtok[:].rearrange("p (c d) -> p c d", d=D),
                    aa_in[s, 0:acols, :].rearrange("(c p) d -> p c d", p=128),
                )
                LIGtok = ap.tile([128, 2 * D], f32, tag="ligtok")
                nc.sync.dma_start(
                    LIGtok[:].rearrange("p (c d) -> p c d", d=D),
                    lig_in[s].rearrange("(c p) d -> p c d", p=128),
                )

                # ---- transposes to feature-major ----
                aaT = ap.tile([128, acols], f32r, tag="aaT")
                for c in range(ta):
                    p = pst.tile([128, 128], f32, tag="tp")
                    nc.tensor.transpose(p[:], AAtok[:, c * D:(c + 1) * D], ident[:])
                    nc.vector.tensor_copy(aaT[:, c * 128:(c + 1) * 128], p[:])
                ligT = ap.tile([128, 2 * 128], f32r, tag="ligT")
                for c in range(2):
                    p = pst.tile([128, 128], f32, tag="tp")
                    nc.tensor.transpose(p[:], LIGtok[:, c * D:(c + 1) * D], ident[:])
                    nc.vector.tensor_copy(ligT[:, c * 128:(c + 1) * 128], p[:])

                # ---- attention 1 projections ----
                qT = ap.tile([128, acols], f32r, tag="qT")
                for off, w in ach:
                    p = ps.tile([128, w], f32, tag="mm")
                    nc.tensor.matmul(p[:], WQ, aaT[:, off:off + w])
                    nc.scalar.activation(qT[:, off:off + w], p[:], AF.Identity, bias=BQ)
                kT = ap.tile([128, 256], f32r, tag="kT")
                p = ps.tile([128, 256], f32, tag="mm")
                nc.tensor.matmul(p[:], WK, ligT[:])
                nc.scalar.activation(kT[:], p[:], AF.Identity, bias=BK)
                vT = ap.tile([128, 256], f32, tag="vT")
                p = ps.tile([128, 256], f32, tag="mm")
                nc.tensor.matmul(p[:], WV, ligT[:])
                nc.scalar.activation(vT[:], p[:], AF.Identity, bias=BV)
                Vtok = ap.tile([128, 2 * D], f32r, tag="Vtok")
                for c in range(2):
                    p = pst.tile([128, 128], f32, tag="tp")
                    nc.tensor.transpose(p[:], vT[:, c * D:(c + 1) * D], ident[:])
                    nc.vector.tensor_scalar(
                        Vtok[:, c * D:(c + 1) * D], p[:],
                        lmt[:, 2 * s + c:2 * s + c + 1], None, OP.mult,
                    )

                # ---- attention 1 scores + message ----
                ST = ap.tile([128, 2 * acols], f32r, tag="ST")
                for lc in range(2):
                    for off, w in ach:
                        p = ps.tile([128, w], f32, tag="mm")
                        nc.tensor.matmul(
                            p[:], kT[:, lc * 128:(lc + 1) * 128], qT[:, off:off + w]
                        )
                        nc.scalar.activation(
                            ST[:, lc * acols + off:lc * acols + off + w], p[:], AF.Sigmoid
                        )
                msgT = ap.tile([128, acols], f32r, tag="msgT")
                for off, w in ach:
                    p = ps.tile([128, w], f32, tag="mm")
                    for lc in range(2):
                        nc.tensor.matmul(
                            p[:], Vtok[:, lc * D:(lc + 1) * D],
                            ST[:, lc * acols + off:lc * acols + off + w],
                            start=(lc == 0), stop=(lc == 1),
                        )
                    nc.vector.tensor_copy(msgT[:, off:off + w], p[:])

                # ---- aa MLP ----
                hT = ap.tile([128, acols], f32r, tag="hT")
                for off, w in ach:
                    p = ps.tile([128, w], f32, tag="mm")
                    nc.tensor.matmul(p[:], WR1, msgT[:, off:off + w])
                    nc.scalar.activation(
                        hT[:, off:off + w], p[:], AF.Lrelu, bias=BR1, alpha=0.1
                    )
                mT = ap.tile([128, acols], f32, tag="mT")
                for off, w in ach:
                    p = ps.tile([128, w], f32, tag="mm")
                    nc.tensor.matmul(p[:], WR2, hT[:, off:off + w])
                    # Lrelu(x/n + b/n) = Lrelu(x + b)/n  (positive homogeneity)
                    nc.scalar.activation(
                        mT[:, off:off + w], p[:], AF.Lrelu,
                        bias=BR2s, scale=INVLs, alpha=0.1,
                    )
                aa_newT = ap.tile([128, acols], f32r, tag="aanewT")
                nc.vector.tensor_tensor(
                    aa_newT[:], mT[:], aaT[:].bitcast(f32), OP.add
                )

                # ---- aa output (token-major, residual from pristine load) ----
                AAout = ap.tile([128, ta * D], f32, tag="aaout")
                for c in range(ta):
                    p = pst.tile([128, 128], f32, tag="tp")
                    nc.tensor.transpose(p[:], mT[:, c * 128:(c + 1) * 128], ident[:])
                    nc.vector.tensor_tensor(
                        AAout[:, c * D:(c + 1) * D], p[:],
                        AAtok[:, c * D:(c + 1) * D], OP.add,
                    )
                    nc.vector.tensor_scalar(
                        AAout[:, c * D:(c + 1) * D], AAout[:, c * D:(c + 1) * D],
                        amt[:, 8 * s + c:8 * s + c + 1], None, OP.mult,
                    )
                nc.sync.dma_start(
                    aa_out[s, 0:acols, :].rearrange("(c p) d -> p c d", p=128),
                    AAout[:].rearrange("p (c d) -> p c d", d=D),
                )

                # ---- attention 2 projections ----
                q2T = ap.tile([128, 256], f32r, tag="q2T")
                p = ps.tile([128, 256], f32, tag="mm")
                nc.tensor.matmul(p[:], WQ, ligT[:])
                nc.scalar.activation(q2T[:], p[:], AF.Identity, bias=BQ)
                k2T = ap.tile([128, acols], f32r, tag="k2T")
                for off, w in ach:
                    p = ps.tile([128, w], f32, tag="mm")
                    nc.tensor.matmul(p[:], WK, aa_newT[:, off:off + w])
                    nc.scalar.activation(k2T[:, off:off + w], p[:], AF.Identity, bias=BK)
                v2T = ap.tile([128, acols], f32, tag="v2T")
                for off, w in ach:
                    p = ps.tile([128, w], f32, tag="mm")
                    nc.tensor.matmul(p[:], WV, aa_newT[:, off:off + w])
                    nc.scalar.activation(v2T[:, off:off + w], p[:], AF.Identity, bias=BV)
                V2tok = ap.tile([128, ta * D], f32r, tag="V2tok")
                for c in range(ta):
                    p = pst.tile([128, 128], f32, tag="tp")
                    nc.tensor.transpose(p[:], v2T[:, c * 128:(c + 1) * 128], ident[:])
                    nc.vector.tensor_scalar(
                        V2tok[:, c * D:(c + 1) * D], p[:],
                        amt[:, 8 * s + c:8 * s + c + 1], None, OP.mult,
                    )

                # ---- attention 2 scores + message ----
                S2T = ap.tile([128, ta * 256], f32r, tag="S2T")
                for ac in range(ta):
                    p = ps.tile([128, 256], f32, tag="mm")
                    nc.tensor.matmul(p[:], k2T[:, ac * 128:(ac + 1) * 128], q2T[:])
                    nc.scalar.activation(
                        S2T[:, ac * 256:(ac + 1) * 256], p[:], AF.Sigmoid
                    )
                msg2T = ap.tile([128, 256], f32r, tag="msg2T")
                p = ps.tile([128, 256], f32, tag="mm")
                for ac in range(ta):
                    nc.tensor.matmul(
                        p[:], V2tok[:, ac * D:(ac + 1) * D],
                        S2T[:, ac * 256:(ac + 1) * 256],
                        start=(ac == 0), stop=(ac == ta - 1),
                    )
                nc.vector.tensor_copy(msg2T[:], p[:])

                # ---- lig MLP ----
                h2T = ap.tile([128, 256], f32r, tag="h2T")
                p = ps.tile([128, 256], f32, tag="mm")
                nc.tensor.matmul(p[:], WL1, msg2T[:])
                nc.scalar.activation(h2T[:], p[:], AF.Lrelu, bias=BL1, alpha=0.1)
                m2T = ap.tile([128, 256], f32, tag="m2T")
                p = ps.tile([128, 256], f32, tag="mm")
                nc.tensor.matmul(p[:], WL2, h2T[:])
                nc.scalar.activation(
                    m2T[:], p[:], AF.Lrelu, bias=BL2s, scale=INVAs, alpha=0.1
                )

                # ---- lig output ----
                LOUT = ap.tile([128, 2 * D], f32, tag="lout")
                for c in range(2):
                    p = pst.tile([128, 128], f32, tag="tp")
                    nc.tensor.transpose(p[:], m2T[:, c * 128:(c + 1) * 128], ident[:])
                    nc.vector.tensor_tensor(
                        LOUT[:, c * D:(c + 1) * D], p[:],
                        LIGtok[:, c * D:(c + 1) * D], OP.add,
                    )
                    nc.vector.tensor_scalar(
                        LOUT[:, c * D:(c + 1) * D], LOUT[:, c * D:(c + 1) * D],
                        lmt[:, 2 * s + c:2 * s + c + 1], None, OP.mult,
                    )
                nc.sync.dma_start(
                    lig_out[s].rearrange("(c p) d -> p c d", p=128),
                    LOUT[:].rearrange("p (c d) -> p c d", d=D),
                )

    nc.compile()
    return nc


class _Runner:
    """Persistent jitted SPMD executor (mirrors bass2jax.run_bass_via_pjrt's
    multi-core branch, but caches the jitted callable across calls)."""

    def __init__(self, nc):
        import jax
        import concourse.mybir as mybir
        from concourse import bass2jax
        from jax.experimental.shard_map import shard_map
        from jax.sharding import Mesh, PartitionSpec

        bass2jax.install_neuronx_cc_hook()
        self.nc = nc
        partition_name = (
            nc.partition_id_tensor.name if nc.partition_id_tensor else None
        )
        in_names, out_names, out_avals = [], [], []
        for alloc in nc.m.functions[0].allocations:
            if not isinstance(alloc, mybir.MemoryLocationSet):
                continue
            name = alloc.memorylocations[0].name
            if alloc.kind == "ExternalInput":
                if name != partition_name:
                    in_names.append(name)
            elif alloc.kind == "ExternalOutput":
                shape = tuple(alloc.tensor_shape)
                out_names.append(name)
                out_avals.append(
                    jax.core.ShapedArray(shape, mybir.dt.np(alloc.dtype))
                )
        self.in_names = list(in_names)
        self.out_names = out_names
        self.out_shapes = [tuple(a.shape) for a in out_avals]
        self.out_dtypes = [a.dtype for a in out_avals]
        n_params = len(in_names)
        all_in_names = in_names + out_names
        if partition_name is not None:
            all_in_names.append(partition_name)
        donate = tuple(range(n_params, n_params + len(out_names)))

        def _body(*args):
            operands = list(args)
            if partition_name is not None:
                operands.append(bass2jax.partition_id_tensor())
            outs = bass2jax._bass_exec_p.bind(
                *operands,
                out_avals=tuple(out_avals),
                in_names=tuple(all_in_names),
                out_names=tuple(out_names),
                lowering_input_output_aliases=(),
                sim_require_finite=True,
                sim_require_nnan=True,
                nc=nc,
            )
            return tuple(outs)

        devices = jax.devices()[:NCORES]
        mesh = Mesh(np.asarray(devices), ("core",))
        in_specs = (PartitionSpec("core"),) * (n_params + len(out_names))
        out_specs = (PartitionSpec("core"),) * len(out_names)
        self.fn = jax.jit(
            shard_map(
                _body, mesh=mesh, in_specs=in_specs, out_specs=out_specs,
                check_rep=False,
            ),
            donate_argnums=donate,
            keep_unused=True,
        )

    def __call__(self, in_maps):
        concat_in = [
            np.concatenate([m[name] for m in in_maps], axis=0)
            for name in self.in_names
        ]
        concat_zeros = [
            np.zeros((NCORES * s[0], *s[1:]), d)
            for s, d in zip(self.out_shapes, self.out_dtypes)
        ]
        out_arrs = self.fn(*concat_in, *concat_zeros)
        return [
            {
                name: np.asarray(out_arrs[i]).reshape(
                    NCORES, *self.out_shapes[i]
                )[c]
                for i, name in enumerate(self.out_names)
            }
            for c in range(NCORES)
        ]


def _get_program(ta_slots):
    key = tuple(ta_slots)
    if key not in _CACHE:
        _CACHE[key] = _Runner(_build(key))
    return _CACHE[key]


def kernel(ligand_features, aa_features, lig_len, aa_len,
           Wq, bq, Wk, bk, Wv, bv, Wr1, br1, Wr2, br2, Wl1, bl1, Wl2, bl2):
    ligand_features = np.ascontiguousarray(np.asarray(ligand_features, dtype=np.float32))
    aa_features = np.ascontiguousarray(np.asarray(aa_features, dtype=np.float32))
    lig_len_np = np.asarray(lig_len).astype(np.int64)
    aa_len_np = np.asarray(aa_len).astype(np.int64)

    # segment -> (core, slot) assignment: sort by aa_len so each slot has
    # near-equal lengths across cores (slot shape = max over its 8 cores)
    order = np.argsort(aa_len_np, kind="stable")
    seg_of = order.reshape(SLOTS, NCORES)  # seg_of[s, c]
    ta_slots = [
        int(-(-int(aa_len_np[seg_of[s]].max()) // 128)) for s in range(SLOTS)
    ]

    runner = _get_program(ta_slots)

    wpack = np.concatenate(
        [np.asarray(w, np.float32) for w in (Wq, Wk, Wv, Wr1, Wr2, Wl1, Wl2)], axis=1
    )
    bcol = np.zeros((D, 8), np.float32)
    for i, b in enumerate((bq, bk, bv, br1, bl1)):
        bcol[:, i] = np.asarray(b, np.float32)
    br2 = np.asarray(br2, np.float32)
    bl2 = np.asarray(bl2, np.float32)

    ar = np.arange(AMAX)
    lr = np.arange(LMAX)

    in_maps = []
    for c in range(NCORES):
        segs = seg_of[:, c]
        nl = lig_len_np[segs].astype(np.float32)
        na = aa_len_np[segs].astype(np.float32)
        scal = np.zeros((D, 4 * SLOTS), np.float32)
        for s in range(SLOTS):
            scal[:, 0 * SLOTS + s] = br2 / nl[s]
            scal[:, 1 * SLOTS + s] = bl2 / na[s]
            scal[:, 2 * SLOTS + s] = 1.0 / nl[s]
            scal[:, 3 * SLOTS + s] = 1.0 / na[s]
        amask = (ar[None, :] < aa_len_np[segs][:, None]).astype(np.float32).ravel()
        lmask = (lr[None, :] < lig_len_np[segs][:, None]).astype(np.float32).ravel()
        in_maps.append({
            "aa_in": np.ascontiguousarray(aa_features[segs]),
            "lig_in": np.ascontiguousarray(ligand_features[segs]),
            "wpack": np.ascontiguousarray(wpack),
            "bcol": bcol,
            "scal": scal,
            "amask": amask,
            "lmask": lmask,
        })

    results = runner(in_maps)

    lig_full = np.zeros((B, LMAX, D), np.float32)
    aa_full = np.zeros((B, AMAX, D), np.float32)
    for c in range(NCORES):
        r = results[c]
        for s in range(SLOTS):
            seg = seg_of[s, c]
            lig_full[seg] = r["lig_out"][s]
            aa_full[seg] = r["aa_out"][s]
    return lig_full, aa_full


if __name__ == "__main__":
    rng = np.random.default_rng(0)
    inputs = dict(
        ligand_features=rng.standard_normal((B, LMAX, D), dtype=np.float32),
        aa_features=rng.standard_normal((B, AMAX, D), dtype=np.float32),
        lig_len=rng.integers(1, LMAX + 1, B).astype(np.int32),
        aa_len=rng.integers(1, AMAX + 1, B).astype(np.int32),
    )
    s = 1.0 / np.sqrt(D)
    for nm in ("Wq", "Wk", "Wv", "Wr1", "Wr2", "Wl1", "Wl2"):
        inputs[nm] = rng.uniform(-s, s, (D, D)).astype(np.float32)
    for nm in ("bq", "bk", "bv", "br1", "br2", "bl1", "bl2"):
        inputs[nm] = rng.uniform(-s, s, D).astype(np.float32)
    lig_o, aa_o = kernel(**inputs)
    print("shapes", lig_o.shape, aa_o.shape)


# revision 12
# speedup vs baseline: 1.4128x; 1.1005x over previous
import sys
import numpy as np

sys.path.insert(0, "/opt/trn_rl_repo")

D = 128
B = 64
LMAX = 256
AMAX = 1024
NCORES = 8
SLOTS = B // NCORES  # 8 segments per core

_CACHE = {}


def _chunks(cols):
    """Split cols (multiple of 128) into pieces <=512, each a multiple of 128,
    preferring pieces >=256 (fp32r full-rate needs moving dim >=256)."""
    out = []
    k = (cols + 511) // 512
    base = cols // k
    base -= base % 128
    rem = cols - base * k
    off = 0
    for i in range(k):
        w = base + (128 if i < rem // 128 else 0)
        out.append((off, w))
        off += w
    assert off == cols, (cols, out)
    return out


def _build(ta_slots, replay=1, probe_identity=False, probe_contig_dma=False,
           probe_no_transpose=False):
    import concourse.bass as bass
    from concourse import bacc
    import concourse.mybir as mybir
    import concourse.tile as tile
    from concourse.masks import make_identity

    dt = mybir.dt
    AF = mybir.ActivationFunctionType
    OP = mybir.AluOpType

    def act(out, in_, func, **kw):
        if probe_identity:
            func = AF.Identity
            kw.pop("alpha", None)
        nc.scalar.activation(out, in_, func, **kw)

    def ptranspose(pdst, src_, ident_):
        if probe_no_transpose:
            nc.vector.tensor_copy(pdst, src_)
        else:
            nc.tensor.transpose(pdst, src_, ident_)

    nc = bacc.Bacc(None)

    aa_in = nc.dram_tensor("aa_in", [SLOTS, AMAX, D], dt.float32, kind="ExternalInput")
    lig_in = nc.dram_tensor("lig_in", [SLOTS, LMAX, D], dt.float32, kind="ExternalInput")
    wpack = nc.dram_tensor("wpack", [D, 7 * D], dt.float32, kind="ExternalInput")
    bcol = nc.dram_tensor("bcol", [D, 8], dt.float32, kind="ExternalInput")
    scal = nc.dram_tensor("scal", [D, 4 * SLOTS], dt.float32, kind="ExternalInput")
    amask = nc.dram_tensor("amask", [SLOTS * AMAX], dt.float32, kind="ExternalInput")
    lmask = nc.dram_tensor("lmask", [SLOTS * LMAX], dt.float32, kind="ExternalInput")
    aa_out = nc.dram_tensor("aa_out", [SLOTS, AMAX, D], dt.float32, kind="ExternalOutput")
    lig_out = nc.dram_tensor("lig_out", [SLOTS, LMAX, D], dt.float32, kind="ExternalOutput")
    scratch = [
        (
            nc.dram_tensor(f"aa_scr{r}", [SLOTS, AMAX, D], dt.float32),
            nc.dram_tensor(f"lig_scr{r}", [SLOTS, LMAX, D], dt.float32),
        )
        for r in range(1, replay)
    ]

    f32 = dt.float32
    f32r = dt.float32r

    with tile.TileContext(nc) as tc:
        with (
            tc.tile_pool(name="const", bufs=1) as cpool,
            tc.tile_pool(name="io", bufs=2) as iop,
            tc.tile_pool(name="act", bufs=2) as ap,
            tc.tile_pool(name="ps", bufs=6, space="PSUM") as ps,
            tc.tile_pool(name="pt", bufs=2, space="PSUM") as pst,
        ):
            # ---- constants ----
            ident = cpool.tile([128, 128], f32)
            make_identity(nc, ident[:])
            wt = cpool.tile([D, 7 * D], f32r)
            nc.sync.dma_start(wt[:], wpack[:].bitcast(f32r))
            WQ = wt[:, 0 * D:1 * D]
            WK = wt[:, 1 * D:2 * D]
            WV = wt[:, 2 * D:3 * D]
            WR1 = wt[:, 3 * D:4 * D]
            WR2 = wt[:, 4 * D:5 * D]
            WL1 = wt[:, 5 * D:6 * D]
            WL2 = wt[:, 6 * D:7 * D]
            bt = cpool.tile([D, 8], f32)
            nc.sync.dma_start(bt[:], bcol[:])
            BQ = bt[:, 0:1]
            BK = bt[:, 1:2]
            BV = bt[:, 2:3]
            BR1 = bt[:, 3:4]
            BL1 = bt[:, 4:5]
            st = cpool.tile([D, 4 * SLOTS], f32)
            nc.sync.dma_start(st[:], scal[:])
            # col layout: [br2/n_l | bl2/n_a | 1/n_l | 1/n_a] per slot
            amt = cpool.tile([128, SLOTS * (AMAX // 128)], f32)
            nc.sync.dma_start(amt[:], amask.rearrange("(n p) -> p n", p=128))
            lmt = cpool.tile([128, SLOTS * (LMAX // 128)], f32)
            nc.sync.dma_start(lmt[:], lmask.rearrange("(n p) -> p n", p=128))

            for rp, s in [(r, q) for r in range(replay) for q in range(SLOTS)]:
                ao, lo = (aa_out, lig_out) if rp == 0 else scratch[rp - 1]
                ta = ta_slots[s]
                acols = ta * 128
                ach = _chunks(acols)
                BR2s = st[:, 0 * SLOTS + s:0 * SLOTS + s + 1]
                BL2s = st[:, 1 * SLOTS + s:1 * SLOTS + s + 1]
                INVLs = st[:, 2 * SLOTS + s:2 * SLOTS + s + 1]
                INVAs = st[:, 3 * SLOTS + s:3 * SLOTS + s + 1]

                # ---- loads (token-major) ----
                AAtok = ap.tile([128, ta * D], f32, tag="aatok")
                if probe_contig_dma:
                    nc.sync.dma_start(
                        AAtok[:],
                        aa_in[s].rearrange("(p c) d -> p (c d)", p=128)[:, 0:ta * D],
                    )
                else:
                    nc.sync.dma_start(
                        AAtok[:].rearrange("p (c d) -> p c d", d=D),
                        aa_in[s, 0:acols, :].rearrange("(c p) d -> p c d", p=128),
                    )
                LIGtok = ap.tile([128, 2 * D], f32, tag="ligtok")
                if probe_contig_dma:
                    nc.sync.dma_start(
                        LIGtok[:],
                        lig_in[s].rearrange("(p c) d -> p (c d)", p=128)[:, 0:2 * D],
                    )
                else:
                    nc.sync.dma_start(
                        LIGtok[:].rearrange("p (c d) -> p c d", d=D),
                        lig_in[s].rearrange("(c p) d -> p c d", p=128),
                    )

                # ---- transposes to feature-major ----
                aaT = ap.tile([128, acols], f32r, tag="aaT")
                for c in range(ta):
                    p = pst.tile([128, 128], f32, tag="tp")
                    ptranspose(p[:], AAtok[:, c * D:(c + 1) * D], ident[:])
                    nc.vector.tensor_copy(aaT[:, c * 128:(c + 1) * 128], p[:])
                ligT = ap.tile([128, 2 * 128], f32r, tag="ligT")
                for c in range(2):
                    p = pst.tile([128, 128], f32, tag="tp")
                    ptranspose(p[:], LIGtok[:, c * D:(c + 1) * D], ident[:])
                    nc.vector.tensor_copy(ligT[:, c * 128:(c + 1) * 128], p[:])

                # ---- attention 1 projections ----
                qT = ap.tile([128, acols], f32r, tag="qT")
                for off, w in ach:
                    p = ps.tile([128, w], f32, tag="mm")
                    nc.tensor.matmul(p[:], WQ, aaT[:, off:off + w])
                    act(qT[:, off:off + w], p[:], AF.Identity, bias=BQ)
                kT = ap.tile([128, 256], f32r, tag="kT")
                p = ps.tile([128, 256], f32, tag="mm")
                nc.tensor.matmul(p[:], WK, ligT[:])
                act(kT[:], p[:], AF.Identity, bias=BK)
                vT = ap.tile([128, 256], f32, tag="vT")
                p = ps.tile([128, 256], f32, tag="mm")
                nc.tensor.matmul(p[:], WV, ligT[:])
                act(vT[:], p[:], AF.Identity, bias=BV)
                Vtok = ap.tile([128, 2 * D], f32r, tag="Vtok")
                for c in range(2):
                    p = pst.tile([128, 128], f32, tag="tp")
                    ptranspose(p[:], vT[:, c * D:(c + 1) * D], ident[:])
                    nc.vector.tensor_scalar(
                        Vtok[:, c * D:(c + 1) * D], p[:],
                        lmt[:, 2 * s + c:2 * s + c + 1], None, OP.mult,
                    )

                # ---- attention 1 scores + message ----
                ST = ap.tile([128, 2 * acols], f32r, tag="ST")
                for lc in range(2):
                    for off, w in ach:
                        p = ps.tile([128, w], f32, tag="mm")
                        nc.tensor.matmul(
                            p[:], kT[:, lc * 128:(lc + 1) * 128], qT[:, off:off + w]
                        )
                        act(
                            ST[:, lc * acols + off:lc * acols + off + w], p[:], AF.Sigmoid
                        )
                msgT = ap.tile([128, acols], f32r, tag="msgT")
                for off, w in ach:
                    p = ps.tile([128, w], f32, tag="mm")
                    for lc in range(2):
                        nc.tensor.matmul(
                            p[:], Vtok[:, lc * D:(lc + 1) * D],
                            ST[:, lc * acols + off:lc * acols + off + w],
                            start=(lc == 0), stop=(lc == 1),
                        )
                    nc.vector.tensor_copy(msgT[:, off:off + w], p[:])

                # ---- aa MLP ----
                hT = ap.tile([128, acols], f32r, tag="hT")
                for off, w in ach:
                    p = ps.tile([128, w], f32, tag="mm")
                    nc.tensor.matmul(p[:], WR1, msgT[:, off:off + w])
                    act(
                        hT[:, off:off + w], p[:], AF.Prelu, bias=BR1, alpha=0.1
                    )
                mT = ap.tile([128, acols], f32, tag="mT")
                for off, w in ach:
                    p = ps.tile([128, w], f32, tag="mm")
                    nc.tensor.matmul(p[:], WR2, hT[:, off:off + w])
                    # Lrelu(x/n + b/n) = Lrelu(x + b)/n  (positive homogeneity)
                    act(
                        mT[:, off:off + w], p[:], AF.Prelu,
                        bias=BR2s, scale=INVLs, alpha=0.1,
                    )
                aa_newT = ap.tile([128, acols], f32r, tag="aanewT")
                nc.vector.tensor_tensor(
                    aa_newT[:], mT[:], aaT[:].bitcast(f32), OP.add
                )

                # ---- aa output (token-major, residual from pristine load) ----
                AAout = ap.tile([128, ta * D], f32, tag="aaout")
                for c in range(ta):
                    p = pst.tile([128, 128], f32, tag="tp")
                    ptranspose(p[:], mT[:, c * 128:(c + 1) * 128], ident[:])
                    nc.vector.tensor_tensor(
                        AAout[:, c * D:(c + 1) * D], p[:],
                        AAtok[:, c * D:(c + 1) * D], OP.add,
                    )
                    nc.vector.tensor_scalar(
                        AAout[:, c * D:(c + 1) * D], AAout[:, c * D:(c + 1) * D],
                        amt[:, 8 * s + c:8 * s + c + 1], None, OP.mult,
                    )
                if probe_contig_dma:
                    nc.sync.dma_start(
                        ao[s].rearrange("(p c) d -> p (c d)", p=128)[:, 0:ta * D],
                        AAout[:],
                    )
                else:
                    nc.sync.dma_start(
                        ao[s, 0:acols, :].rearrange("(c p) d -> p c d", p=128),
                        AAout[:].rearrange("p (c d) -> p c d", d=D),
                    )

                # ---- attention 2 projections ----
                q2T = ap.tile([128, 256], f32r, tag="q2T")
                p = ps.tile([128, 256], f32, tag="mm")
                nc.tensor.matmul(p[:], WQ, ligT[:])
                act(q2T[:], p[:], AF.Identity, bias=BQ)
                k2T = ap.tile([128, acols], f32r, tag="k2T")
                for off, w in ach:
                    p = ps.tile([128, w], f32, tag="mm")
                    nc.tensor.matmul(p[:], WK, aa_newT[:, off:off + w])
                    act(k2T[:, off:off + w], p[:], AF.Identity, bias=BK)
                v2T = ap.tile([128, acols], f32, tag="v2T")
                for off, w in ach:
                    p = ps.tile([128, w], f32, tag="mm")
                    nc.tensor.matmul(p[:], WV, aa_newT[:, off:off + w])
                    act(v2T[:, off:off + w], p[:], AF.Identity, bias=BV)
                V2tok = ap.tile([128, ta * D], f32r, tag="V2tok")
                for c in range(ta):
                    p = pst.tile([128, 128], f32, tag="tp")
                    ptranspose(p[:], v2T[:, c * 128:(c + 1) * 128], ident[:])
                    nc.vector.tensor_scalar(
                        V2tok[:, c * D:(c + 1) * D], p[:],
                        amt[:, 8 * s + c:8 * s + c + 1], None, OP.mult,
                    )

                # ---- attention 2 scores + message ----
                S2T = ap.tile([128, ta * 256], f32r, tag="S2T")
                for ac in range(ta):
                    p = ps.tile([128, 256], f32, tag="mm")
                    nc.tensor.matmul(p[:], k2T[:, ac * 128:(ac + 1) * 128], q2T[:])
                    act(
                        S2T[:, ac * 256:(ac + 1) * 256], p[:], AF.Sigmoid
                    )
                msg2T = ap.tile([128, 256], f32r, tag="msg2T")
                p = ps.tile([128, 256], f32, tag="mm")
                for ac in range(ta):
                    nc.tensor.matmul(
                        p[:], V2tok[:, ac * D:(ac + 1) * D],
                        S2T[:, ac * 256:(ac + 1) * 256],
                        start=(ac == 0), stop=(ac == ta - 1),
                    )
                nc.vector.tensor_copy(msg2T[:], p[:])

                # ---- lig MLP ----
                h2T = ap.tile([128, 256], f32r, tag="h2T")
                p = ps.tile([128, 256], f32, tag="mm")
                nc.tensor.matmul(p[:], WL1, msg2T[:])
                act(h2T[:], p[:], AF.Prelu, bias=BL1, alpha=0.1)
                m2T = ap.tile([128, 256], f32, tag="m2T")
                p = ps.tile([128, 256], f32, tag="mm")
                nc.tensor.matmul(p[:], WL2, h2T[:])
                act(
                    m2T[:], p[:], AF.Prelu, bias=BL2s, scale=INVAs, alpha=0.1
                )

                # ---- lig output ----
                LOUT = ap.tile([128, 2 * D], f32, tag="lout")
                for c in range(2):
                    p = pst.tile([128, 128], f32, tag="tp")
                    ptranspose(p[:], m2T[:, c * 128:(c + 1) * 128], ident[:])
                    nc.vector.tensor_tensor(
                        LOUT[:, c * D:(c + 1) * D], p[:],
                        LIGtok[:, c * D:(c + 1) * D], OP.add,
                    )
                    nc.vector.tensor_scalar(
                        LOUT[:, c * D:(c + 1) * D], LOUT[:, c * D:(c + 1) * D],
                        lmt[:, 2 * s + c:2 * s + c + 1], None, OP.mult,
                    )
                if probe_contig_dma:
                    nc.sync.dma_start(
                        lo[s].rearrange("(p c) d -> p (c d)", p=128)[:, 0:2 * D],
                        LOUT[:],
                    )
                else:
                    nc.sync.dma_start(
                        lo[s].rearrange("(c p) d -> p c d", p=128),
                        LOUT[:].rearrange("p (c d) -> p c d", d=D),
                    )

    nc.compile()
    return nc


class _Runner:
    """Persistent jitted SPMD executor (mirrors bass2jax.run_bass_via_pjrt's
    multi-core branch, but caches the jitted callable across calls)."""

    def __init__(self, nc):
        import jax
        import concourse.mybir as mybir
        from concourse import bass2jax
        from jax.experimental.shard_map import shard_map
        from jax.sharding import Mesh, PartitionSpec

        bass2jax.install_neuronx_cc_hook()
        self.nc = nc
        partition_name = (
            nc.partition_id_tensor.name if nc.partition_id_tensor else None
        )
        in_names, out_names, out_avals = [], [], []
        for alloc in nc.m.functions[0].allocations:
            if not isinstance(alloc, mybir.MemoryLocationSet):
                continue
            name = alloc.memorylocations[0].name
            if alloc.kind == "ExternalInput":
                if name != partition_name:
                    in_names.append(name)
            elif alloc.kind == "ExternalOutput":
                shape = tuple(alloc.tensor_shape)
                out_names.append(name)
                out_avals.append(
                    jax.core.ShapedArray(shape, mybir.dt.np(alloc.dtype))
                )
        self.in_names = list(in_names)
        self.out_names = out_names
        self.out_shapes = [tuple(a.shape) for a in out_avals]
        self.out_dtypes = [a.dtype for a in out_avals]
        n_params = len(in_names)
        all_in_names = in_names + out_names
        if partition_name is not None:
            all_in_names.append(partition_name)
        donate = tuple(range(n_params, n_params + len(out_names)))

        def _body(*args):
            operands = list(args)
            if partition_name is not None:
                operands.append(bass2jax.partition_id_tensor())
            outs = bass2jax._bass_exec_p.bind(
                *operands,
                out_avals=tuple(out_avals),
                in_names=tuple(all_in_names),
                out_names=tuple(out_names),
                lowering_input_output_aliases=(),
                sim_require_finite=True,
                sim_require_nnan=True,
                nc=nc,
            )
            return tuple(outs)

        devices = jax.devices()[:NCORES]
        mesh = Mesh(np.asarray(devices), ("core",))
        in_specs = (PartitionSpec("core"),) * (n_params + len(out_names))
        out_specs = (PartitionSpec("core"),) * len(out_names)
        self.fn = jax.jit(
            shard_map(
                _body, mesh=mesh, in_specs=in_specs, out_specs=out_specs,
                check_rep=False,
            ),
            donate_argnums=donate,
            keep_unused=True,
        )

    def __call__(self, in_maps):
        concat_in = [
            np.concatenate([m[name] for m in in_maps], axis=0)
            for name in self.in_names
        ]
        concat_zeros = [
            np.zeros((NCORES * s[0], *s[1:]), d)
            for s, d in zip(self.out_shapes, self.out_dtypes)
        ]
        out_arrs = self.fn(*concat_in, *concat_zeros)
        return [
            {
                name: np.asarray(out_arrs[i]).reshape(
                    NCORES, *self.out_shapes[i]
                )[c]
                for i, name in enumerate(self.out_names)
            }
            for c in range(NCORES)
        ]


def _get_program(ta_slots):
    key = tuple(ta_slots)
    if key not in _CACHE:
        _CACHE[key] = _Runner(_build(key))
    return _CACHE[key]


def kernel(ligand_features, aa_features, lig_len, aa_len,
           Wq, bq, Wk, bk, Wv, bv, Wr1, br1, Wr2, br2, Wl1, bl1, Wl2, bl2):
    ligand_features = np.ascontiguousarray(np.asarray(ligand_features, dtype=np.float32))
    aa_features = np.ascontiguousarray(np.asarray(aa_features, dtype=np.float32))
    lig_len_np = np.asarray(lig_len).astype(np.int64)
    aa_len_np = np.asarray(aa_len).astype(np.int64)

    # segment -> (core, slot) assignment: sort by aa_len so each slot has
    # near-equal lengths across cores (slot shape = max over its 8 cores)
    order = np.argsort(aa_len_np, kind="stable")
    seg_of = order.reshape(SLOTS, NCORES)  # seg_of[s, c]
    ta_slots = [
        int(-(-int(aa_len_np[seg_of[s]].max()) // 128)) for s in range(SLOTS)
    ]

    runner = _get_program(ta_slots)

    wpack = np.concatenate(
        [np.asarray(w, np.float32) for w in (Wq, Wk, Wv, Wr1, Wr2, Wl1, Wl2)], axis=1
    )
    bcol = np.zeros((D, 8), np.float32)
    for i, b in enumerate((bq, bk, bv, br1, bl1)):
        bcol[:, i] = np.asarray(b, np.float32)
    br2 = np.asarray(br2, np.float32)
    bl2 = np.asarray(bl2, np.float32)

    ar = np.arange(AMAX)
    lr = np.arange(LMAX)

    in_maps = []
    for c in range(NCORES):
        segs = seg_of[:, c]
        nl = lig_len_np[segs].astype(np.float32)
        na = aa_len_np[segs].astype(np.float32)
        scal = np.zeros((D, 4 * SLOTS), np.float32)
        for s in range(SLOTS):
            scal[:, 0 * SLOTS + s] = br2 / nl[s]
            scal[:, 1 * SLOTS + s] = bl2 / na[s]
            scal[:, 2 * SLOTS + s] = 1.0 / nl[s]
            scal[:, 3 * SLOTS + s] = 1.0 / na[s]
        amask = (ar[None, :] < aa_len_np[segs][:, None]).astype(np.float32).ravel()
        lmask = (lr[None, :] < lig_len_np[segs][:, None]).astype(np.float32).ravel()
        in_maps.append({
            "aa_in": np.ascontiguousarray(aa_features[segs]),
            "lig_in": np.ascontiguousarray(ligand_features[segs]),
            "wpack": np.ascontiguousarray(wpack),
            "bcol": bcol,
            "scal": scal,
            "amask": amask,
            "lmask": lmask,
        })

    results = runner(in_maps)

    lig_full = np.zeros((B, LMAX, D), np.float32)
    aa_full = np.zeros((B, AMAX, D), np.float32)
    for c in range(NCORES):
        r = results[c]
        for s in range(SLOTS):
            seg = seg_of[s, c]
            lig_full[seg] = r["lig_out"][s]
            aa_full[seg] = r["aa_out"][s]
    return lig_full, aa_full


if __name__ == "__main__":
    rng = np.random.default_rng(0)
    inputs = dict(
        ligand_features=rng.standard_normal((B, LMAX, D), dtype=np.float32),
        aa_features=rng.standard_normal((B, AMAX, D), dtype=np.float32),
        lig_len=rng.integers(1, LMAX + 1, B).astype(np.int32),
        aa_len=rng.integers(1, AMAX + 1, B).astype(np.int32),
    )
    s = 1.0 / np.sqrt(D)
    for nm in ("Wq", "Wk", "Wv", "Wr1", "Wr2", "Wl1", "Wl2"):
        inputs[nm] = rng.uniform(-s, s, (D, D)).astype(np.float32)
    for nm in ("bq", "bk", "bv", "br1", "br2", "bl1", "bl2"):
        inputs[nm] = rng.uniform(-s, s, D).astype(np.float32)
    lig_o, aa_o = kernel(**inputs)
    print("shapes", lig_o.shape, aa_o.shape)
